# revision 28
# baseline (speedup 1.0000x reference)
"""MoE layer (top-2 of 8 experts) on 8 TRN2 NeuronCores.

Strategy:
  Phase 1 (device, data-parallel): each core computes gate logits
      logitsT = gate_w.T @ x_shard.T for B/8 tokens (fp32 matmul).
  Host: softmax + top-2 + renormalized weights (the routing / sharding
      decision), build per-expert token index lists, pad to a common
      capacity C (multiple of the token block).
  Phase 2 (device, expert-parallel): core e runs its expert's FFN over
      the tokens routed to it: y = (relu(x@W1+b1)@W2 + b2) * w_token.
      bf16 matmuls, fp32 PSUM accumulation, weights SBUF-resident.
  Host: scatter-add the two scaled contributions per token.
"""

import numpy as np
import ml_dtypes

import concourse.mybir as mybir
import concourse.tile as tile
from concourse import bacc
from concourse.bass_utils import run_bass_kernel_spmd

P = 128
N_CORES = 8
CB = 256  # phase-2 token block
BF16 = mybir.dt.bfloat16
F32 = mybir.dt.float32
_bf16_np = ml_dtypes.bfloat16

_build_cache = {}


def _build_gate(D, E, T):
    """Per-core gate matmul: logitsT[E, T] = gate_w[D, E].T @ xT[D, T]."""
    nc = bacc.Bacc(None, target_bir_lowering=False)
    xT = nc.dram_tensor("xT", [D, T], F32, kind="ExternalInput")
    gw = nc.dram_tensor("gw", [D, E], F32, kind="ExternalInput")
    logitsT = nc.dram_tensor("logitsT", [E, T], F32, kind="ExternalOutput")
    DO = D // P
    NT = 512
    xT_r = xT.rearrange("(do p) t -> p do t", p=P)
    with tile.TileContext(nc) as tc:
        with (
            tc.tile_pool(name="sb", bufs=2) as sb,
            tc.tile_pool(name="consts", bufs=1) as cp,
            tc.tile_pool(name="xp", bufs=2) as xp,
            tc.tile_pool(name="ps", bufs=2, space="PSUM") as ps,
        ):
            gw_sb = cp.tile([P, DO, E], F32, tag="gw")
            nc.sync.dma_start(gw_sb[:], gw.rearrange("(do p) e -> p do e", p=P))
            for tt in range(T // NT):
                # per-(token-tile, d-tile) x chunks (256KB) so the first
                # matmul starts as soon as the first chunk lands
                xdi = []
                for di in range(DO):
                    xt = xp.tile([P, NT], F32, tag=f"x{di}", name=f"x{di}")
                    eng = nc.sync if di % 2 == 0 else nc.scalar
                    eng.dma_start(xt[:], xT_r[:, di, tt * NT:(tt + 1) * NT])
                    xdi.append(xt)
                pt = ps.tile([E, NT], F32, tag="pt")
                for di in range(DO):
                    nc.tensor.matmul(
                        pt[:],
                        gw_sb[:, di],
                        xdi[di][:],
                        start=(di == 0),
                        stop=(di == DO - 1),
                    )
                ot = sb.tile([E, NT], F32, tag="ot")
                nc.vector.tensor_copy(ot[:], pt[:])
                nc.sync.dma_start(logitsT[:, tt * NT:(tt + 1) * NT], ot[:])
    nc.finalize()
    return nc


def _build_expert(D, H, O, C):
    """Per-core expert FFN over C (padded) routed tokens.

    y[C, O] = (relu(x @ W1 + b1) @ W2 + b2) * w_token[:, None]
    computed as hT = W1.T-slices @ xT (keeps H on partitions), then
    y = hT-slices.T @ W2 (tokens back on partitions). No transposes on
    device: xT / b1 / wt come host-prearranged.
    """
    nc = bacc.Bacc(None, target_bir_lowering=False)
    xT = nc.dram_tensor("xT", [D, C], BF16, kind="ExternalInput")
    w1 = nc.dram_tensor("w1", [D, H], BF16, kind="ExternalInput")
    w2 = nc.dram_tensor("w2", [H, O], BF16, kind="ExternalInput")
    b1 = nc.dram_tensor("b1", [P, H // P], F32, kind="ExternalInput")
    b2 = nc.dram_tensor("b2", [P, O], F32, kind="ExternalInput")
    wt = nc.dram_tensor("wt", [P, C // P], F32, kind="ExternalInput")
    y = nc.dram_tensor("y", [C, O], F32, kind="ExternalOutput")
    DO, HO = D // P, H // P
    OO = O // 512
    # token blocks of CB, trailing 128-block if C % CB != 0
    starts = []
    pos = 0
    while pos < C:
        cb = CB if C - pos >= CB else P
        starts.append((pos, cb))
        pos += cb
    # chunk the weight loads so the first matmuls start after ~1MB of DMA;
    # the first two W1 chunks are half-size so L1 starts even earlier
    HC = 4                   # h-tiles (of 128) per W2 weight chunk
    NWC = HO // HC           # number of W2 weight chunks
    w1_chunks = [(0, 2), (2, 2)] + [(h, 4) for h in range(4, HO, 4)]
    w1_of_hi = {}            # hi -> (chunk index, offset within chunk)
    for ci, (h0, nh) in enumerate(w1_chunks):
        for j in range(nh):
            w1_of_hi[h0 + j] = (ci, j)
    HG = 8                   # h-tiles per hT group tile (finer L2 deps)
    NHG = HO // HG
    y_r = y.rearrange("(n p) o -> p n o", p=P)
    w1_r = w1.rearrange("(do p) h -> p do h", p=P)
    w2_r = w2.rearrange("(ho p) o -> p ho o", p=P)
    with tile.TileContext(nc) as tc:
        with (
            tc.tile_pool(name="wpool", bufs=1) as wp,
            tc.tile_pool(name="xpool", bufs=3) as xp,
            tc.tile_pool(name="hpool", bufs=2) as hp,
            tc.tile_pool(name="opool", bufs=4) as op,
            tc.tile_pool(name="hps", bufs=3, space="PSUM") as hps,
            tc.tile_pool(name="yps", bufs=3, space="PSUM") as yps,
        ):
            xT_r = xT.rearrange("(do p) c -> p do c", p=P)
            # startup-critical DMAs: W1 chunks in consumption order on the
            # sync ring (L1 of block 0 chases W1's delivery); block-0 x,
            # W2 chunk 0 and biases on the scalar HWDGE ring.
            x0_sb = xp.tile([P, DO, CB], BF16, tag="x")
            nc.scalar.dma_start(x0_sb[:, :, :starts[0][1]], xT_r[:, :, 0:starts[0][1]])
            w1c = [wp.tile([P, DO, nh * P], BF16, tag=f"w1_{k}", name=f"w1_{k}")
                   for k, (h0, nh) in enumerate(w1_chunks)]
            w2c = [wp.tile([P, HC, O], BF16, tag=f"w2_{k}", name=f"w2_{k}") for k in range(NWC)]
            for k, (h0, nh) in enumerate(w1_chunks):
                nc.sync.dma_start(w1c[k][:], w1_r[:, :, h0 * P:(h0 + nh) * P])
            b1_sb = wp.tile([P, HO], F32, tag="b1")
            nc.scalar.dma_start(b1_sb[:], b1[:])
            nc.scalar.dma_start(w2c[0][:], w2_r[:, 0:HC])
            b2_sb = wp.tile([P, O], F32, tag="b2")
            nc.scalar.dma_start(b2_sb[:], b2[:])
            wt_sb = wp.tile([P, C // P], F32, tag="wt")
            nc.scalar.dma_start(wt_sb[:], wt[:])

            # W2 chunks 1.. are paced behind block-0 relus so they don't
            # race the critical W1 stream during startup
            w2_load_after = {
                4 * k: [(w2c[k], w2_r[:, k * HC:(k + 1) * HC])]
                for k in range(1, NWC)
            }
            for blk, (n0, cb) in enumerate(starts):
                if blk == 0:
                    x_sb = x0_sb[:, :, :cb]
                else:
                    x_sb = xp.tile([P, DO, CB], BF16, tag="x", name="x_sb")[:, :, :cb]
                    nc.sync.dma_start(x_sb[:], xT_r[:, :, n0:n0 + cb])
                hgs = [hp.tile([P, HG, CB], BF16, tag=f"h{g}", name=f"h{g}")[:, :, :cb]
                       for g in range(NHG)]
                for hi in range(HO):
                    ph = hps.tile([P, CB], F32, tag="ph", name="ph")[:, :cb]
                    ci, off = w1_of_hi[hi]
                    for di in range(DO):
                        nc.tensor.matmul(
                            ph[:],
                            w1c[ci][:, di, off * P:(off + 1) * P],
                            x_sb[:, di],
                            start=(di == 0),
                            stop=(di == DO - 1),
                        )
                    act = nc.scalar.activation(
                        hgs[hi // HG][:, hi % HG], ph[:],
                        mybir.ActivationFunctionType.Relu,
                        bias=b1_sb[:, hi:hi + 1],
                    )
                    if blk == 0 and hi in w2_load_after:
                        # W2 chunk k streams only after L1 consumed W1 chunk
                        # k, so it never races the critical W1 delivery
                        for w2t, w2src in w2_load_after[hi]:
                            dma = nc.scalar.dma_start(w2t[:], w2src)
                            tile.add_dep_helper(
                                dma.ins, act.ins,
                                reason="pace late load behind W1 consumption",
                            )
                for ct in range(cb // P):
                    # hi outer / ot inner: both ot matmuls share the same
                    # stationary hT slice, halving LDWEIGHTS pressure
                    yps_ct = [yps.tile([P, 512], F32, tag="yp", name="yp")
                              for _ in range(OO)]
                    for hi in range(HO):
                        for ot in range(OO):
                            nc.tensor.matmul(
                                yps_ct[ot][:],
                                hgs[hi // HG][:, hi % HG, ct * P:(ct + 1) * P],
                                w2c[hi // HC][:, hi % HC, ot * 512:(ot + 1) * 512],
                                start=(hi == 0),
                                stop=(hi == HO - 1),
                            )
                    for ot in range(OO):
                        o_sb = op.tile([P, 512], F32, tag="o")
                        nc.vector.tensor_add(
                            o_sb[:], yps_ct[ot][:], b2_sb[:, ot * 512:(ot + 1) * 512]
                        )
                        n_idx = n0 // P + ct
                        nc.vector.tensor_scalar_mul(
                            o_sb[:], o_sb[:], wt_sb[:, n_idx:n_idx + 1]
                        )
                        nc.sync.dma_start(
                            y_r[:, n_idx, ot * 512:(ot + 1) * 512], o_sb[:]
                        )
    nc.finalize()
    return nc


def kernel(x, W1, b1, W2, b2, gate_w, gate_b):
    x = np.ascontiguousarray(x, dtype=np.float32)
    W1 = np.asarray(W1, dtype=np.float32)
    b1 = np.asarray(b1, dtype=np.float32)
    W2 = np.asarray(W2, dtype=np.float32)
    b2 = np.asarray(b2, dtype=np.float32)
    gate_w = np.ascontiguousarray(gate_w, dtype=np.float32)
    gate_b = np.asarray(gate_b, dtype=np.float32)

    B, D = x.shape
    E, _, H = W1.shape
    O = W2.shape[2]
    assert E == N_CORES and B % (N_CORES * 512) == 0 and D % P == 0
    T = B // N_CORES
    core_ids = list(range(N_CORES))

    # ---- Phase 1: gate logits on device (data-parallel over tokens) ----
    key = ("gate", D, E, T)
    if key not in _build_cache:
        _build_cache[key] = _build_gate(D, E, T)
    nc_gate = _build_cache[key]
    in_maps = [
        {"xT": np.ascontiguousarray(x[i * T:(i + 1) * T].T), "gw": gate_w}
        for i in range(N_CORES)
    ]
    res = run_bass_kernel_spmd(nc_gate, in_maps, core_ids=core_ids)
    logits = np.concatenate(
        [res.results[i]["logitsT"].T for i in range(N_CORES)], axis=0
    ) + gate_b[None, :]

    # ---- Host: top-2 routing (the expert-parallel sharding decision) ----
    lg = logits.astype(np.float64)
    lg -= lg.max(axis=1, keepdims=True)
    probs = np.exp(lg)
    probs /= probs.sum(axis=1, keepdims=True)
    order = np.argsort(-probs, axis=1, kind="stable")[:, :2]
    p_top = np.take_along_axis(probs, order, axis=1)
    w_top = p_top / p_top.sum(axis=1, keepdims=True)  # [B, 2]

    idx_e, wt_e = [], []
    for e in range(E):
        m0 = order[:, 0] == e
        m1 = order[:, 1] == e
        sel = m0 | m1
        idx = np.nonzero(sel)[0]
        w = np.where(m0[sel], w_top[sel, 0], w_top[sel, 1]).astype(np.float32)
        idx_e.append(idx)
        wt_e.append(w)
    max_count = max(len(i) for i in idx_e)
    C = max(CB, ((max_count + P - 1) // P) * P)

    # ---- Phase 2: expert FFN on device (expert-parallel) ----
    key = ("expert", D, H, O, C)
    if key not in _build_cache:
        _build_cache[key] = _build_expert(D, H, O, C)
    nc_exp = _build_cache[key]

    in_maps = []
    for e in range(E):
        n_e = len(idx_e[e])
        xT_pad = np.zeros((D, C), dtype=_bf16_np)
        xT_pad[:, :n_e] = x[idx_e[e]].T.astype(_bf16_np)
        wt_pad = np.zeros(C, dtype=np.float32)
        wt_pad[:n_e] = wt_e[e]
        in_maps.append({
            "xT": xT_pad,
            "w1": W1[e].astype(_bf16_np),
            "w2": W2[e].astype(_bf16_np),
            "b1": np.ascontiguousarray(b1[e].reshape(H // P, P).T),
            "b2": np.ascontiguousarray(np.broadcast_to(b2[e], (P, O))),
            "wt": np.ascontiguousarray(wt_pad.reshape(C // P, P).T),
        })
    res = run_bass_kernel_spmd(nc_exp, in_maps, core_ids=core_ids)

    # ---- Host: un-permute and combine the two expert contributions ----
    out = np.zeros((B, O), dtype=np.float32)
    for e in range(E):
        n_e = len(idx_e[e])
        if n_e:
            out[idx_e[e]] += res.results[e]["y"][:n_e]
    return out


# revision 29
# speedup vs baseline: 1.0052x; 1.0052x over previous
"""MoE layer (top-2 of 8 experts) on 8 TRN2 NeuronCores.

Strategy:
  Phase 1 (device, data-parallel): each core computes gate logits
      logitsT = gate_w.T @ x_shard.T for B/8 tokens (fp32 matmul).
  Host: softmax + top-2 + renormalized weights (the routing / sharding
      decision), build per-expert token index lists, pad to a common
      capacity C (multiple of the token block).
  Phase 2 (device, expert-parallel): core e runs its expert's FFN over
      the tokens routed to it: y = (relu(x@W1+b1)@W2 + b2) * w_token.
      bf16 matmuls, fp32 PSUM accumulation, weights SBUF-resident.
  Host: scatter-add the two scaled contributions per token.
"""

import numpy as np
import ml_dtypes

import concourse.mybir as mybir
import concourse.tile as tile
from concourse import bacc
from concourse.bass_utils import run_bass_kernel_spmd

P = 128
N_CORES = 8
CB = 256  # phase-2 token block
BF16 = mybir.dt.bfloat16
F32 = mybir.dt.float32
_bf16_np = ml_dtypes.bfloat16

_build_cache = {}


def _build_gate(D, E, T):
    """Per-core gate matmul: logitsT[E, T] = gate_w[D, E].T @ xT[D, T]."""
    nc = bacc.Bacc(None, target_bir_lowering=False)
    xT = nc.dram_tensor("xT", [D, T], F32, kind="ExternalInput")
    gw = nc.dram_tensor("gw", [D, E], F32, kind="ExternalInput")
    logitsT = nc.dram_tensor("logitsT", [E, T], F32, kind="ExternalOutput")
    DO = D // P
    NT = 512
    xT_r = xT.rearrange("(do p) t -> p do t", p=P)
    with tile.TileContext(nc) as tc:
        with (
            tc.tile_pool(name="sb", bufs=2) as sb,
            tc.tile_pool(name="consts", bufs=1) as cp,
            tc.tile_pool(name="xp", bufs=2) as xp,
            tc.tile_pool(name="ps", bufs=2, space="PSUM") as ps,
        ):
            gw_sb = cp.tile([P, DO, E], F32, tag="gw")
            nc.sync.dma_start(gw_sb[:], gw.rearrange("(do p) e -> p do e", p=P))
            for tt in range(T // NT):
                # per-(token-tile, d-tile) x chunks (256KB) so the first
                # matmul starts as soon as the first chunk lands
                xdi = []
                for di in range(DO):
                    xt = xp.tile([P, NT], F32, tag=f"x{di}", name=f"x{di}")
                    eng = nc.sync if di % 2 == 0 else nc.scalar
                    eng.dma_start(xt[:], xT_r[:, di, tt * NT:(tt + 1) * NT])
                    xdi.append(xt)
                pt = ps.tile([E, NT], F32, tag="pt")
                for di in range(DO):
                    nc.tensor.matmul(
                        pt[:],
                        gw_sb[:, di],
                        xdi[di][:],
                        start=(di == 0),
                        stop=(di == DO - 1),
                    )
                ot = sb.tile([E, NT], F32, tag="ot")
                nc.vector.tensor_copy(ot[:], pt[:])
                nc.sync.dma_start(logitsT[:, tt * NT:(tt + 1) * NT], ot[:])
    nc.finalize()
    return nc


def _build_expert(D, H, O, C):
    """Per-core expert FFN over C (padded) routed tokens.

    y[C, O] = (relu(x @ W1 + b1) @ W2 + b2) * w_token[:, None]
    computed as hT = W1.T-slices @ xT (keeps H on partitions), then
    y = hT-slices.T @ W2 (tokens back on partitions). No transposes on
    device: xT / b1 / wt come host-prearranged.
    """
    nc = bacc.Bacc(None, target_bir_lowering=False)
    xT = nc.dram_tensor("xT", [D, C], BF16, kind="ExternalInput")
    w1 = nc.dram_tensor("w1", [D, H], BF16, kind="ExternalInput")
    w2 = nc.dram_tensor("w2", [H, O], BF16, kind="ExternalInput")
    b1 = nc.dram_tensor("b1", [P, H // P], F32, kind="ExternalInput")
    b2 = nc.dram_tensor("b2", [P, O], F32, kind="ExternalInput")
    wt = nc.dram_tensor("wt", [P, C // P], F32, kind="ExternalInput")
    y = nc.dram_tensor("y", [C, O], F32, kind="ExternalOutput")
    DO, HO = D // P, H // P
    OO = O // 512
    # token blocks of CB, trailing 128-block if C % CB != 0
    starts = []
    pos = 0
    while pos < C:
        cb = CB if C - pos >= CB else P
        starts.append((pos, cb))
        pos += cb
    # chunk the weight loads so the first matmuls start after ~1MB of DMA;
    # the first two W1 chunks are half-size so L1 starts even earlier
    HC = 4                   # h-tiles (of 128) per W2 weight chunk
    NWC = HO // HC           # number of W2 weight chunks
    w1_chunks = [(0, 2), (2, 2)] + [(h, 4) for h in range(4, HO, 4)]
    w1_of_hi = {}            # hi -> (chunk index, offset within chunk)
    for ci, (h0, nh) in enumerate(w1_chunks):
        for j in range(nh):
            w1_of_hi[h0 + j] = (ci, j)
    HG = 8                   # h-tiles per hT group tile (finer L2 deps)
    NHG = HO // HG
    y_r = y.rearrange("(n p) o -> p n o", p=P)
    w1_r = w1.rearrange("(do p) h -> p do h", p=P)
    w2_r = w2.rearrange("(ho p) o -> p ho o", p=P)
    with tile.TileContext(nc) as tc:
        with (
            tc.tile_pool(name="wpool", bufs=1) as wp,
            tc.tile_pool(name="xpool", bufs=3) as xp,
            tc.tile_pool(name="hpool", bufs=2) as hp,
            tc.tile_pool(name="opool", bufs=4) as op,
            tc.tile_pool(name="hps", bufs=4, space="PSUM") as hps,
            tc.tile_pool(name="yps", bufs=3, space="PSUM") as yps,
        ):
            xT_r = xT.rearrange("(do p) c -> p do c", p=P)
            # startup-critical DMAs: W1 chunks in consumption order on the
            # sync ring (L1 of block 0 chases W1's delivery); block-0 x,
            # W2 chunk 0 and biases on the scalar HWDGE ring.
            x0_sb = xp.tile([P, DO, CB], BF16, tag="x")
            nc.scalar.dma_start(x0_sb[:, :, :starts[0][1]], xT_r[:, :, 0:starts[0][1]])
            w1c = [wp.tile([P, DO, nh * P], BF16, tag=f"w1_{k}", name=f"w1_{k}")
                   for k, (h0, nh) in enumerate(w1_chunks)]
            w2c = [wp.tile([P, HC, O], BF16, tag=f"w2_{k}", name=f"w2_{k}") for k in range(NWC)]
            for k, (h0, nh) in enumerate(w1_chunks):
                nc.sync.dma_start(w1c[k][:], w1_r[:, :, h0 * P:(h0 + nh) * P])
            b1_sb = wp.tile([P, HO], F32, tag="b1")
            nc.scalar.dma_start(b1_sb[:], b1[:])
            nc.scalar.dma_start(w2c[0][:], w2_r[:, 0:HC])
            b2_sb = wp.tile([P, O], F32, tag="b2")
            nc.scalar.dma_start(b2_sb[:], b2[:])
            wt_sb = wp.tile([P, C // P], F32, tag="wt")
            nc.scalar.dma_start(wt_sb[:], wt[:])

            # W2 chunks 1.. are paced behind block-0 relus so they don't
            # race the critical W1 stream during startup
            w2_load_after = {
                4 * k: [(w2c[k], w2_r[:, k * HC:(k + 1) * HC])]
                for k in range(1, NWC)
            }
            for blk, (n0, cb) in enumerate(starts):
                if blk == 0:
                    x_sb = x0_sb[:, :, :cb]
                else:
                    x_sb = xp.tile([P, DO, CB], BF16, tag="x", name="x_sb")[:, :, :cb]
                    nc.sync.dma_start(x_sb[:], xT_r[:, :, n0:n0 + cb])
                hgs = [hp.tile([P, HG, CB], BF16, tag=f"h{g}", name=f"h{g}")[:, :, :cb]
                       for g in range(NHG)]
                for hi in range(HO):
                    ph = hps.tile([P, CB], F32, tag="ph", name="ph")[:, :cb]
                    ci, off = w1_of_hi[hi]
                    for di in range(DO):
                        nc.tensor.matmul(
                            ph[:],
                            w1c[ci][:, di, off * P:(off + 1) * P],
                            x_sb[:, di],
                            start=(di == 0),
                            stop=(di == DO - 1),
                        )
                    act = nc.scalar.activation(
                        hgs[hi // HG][:, hi % HG], ph[:],
                        mybir.ActivationFunctionType.Relu,
                        bias=b1_sb[:, hi:hi + 1],
                    )
                    if blk == 0 and hi in w2_load_after:
                        # W2 chunk k streams only after L1 consumed W1 chunk
                        # k, so it never races the critical W1 delivery
                        for w2t, w2src in w2_load_after[hi]:
                            dma = nc.scalar.dma_start(w2t[:], w2src)
                            tile.add_dep_helper(
                                dma.ins, act.ins,
                                reason="pace late load behind W1 consumption",
                            )
                for ct in range(cb // P):
                    # hi outer / ot inner: both ot matmuls share the same
                    # stationary hT slice, halving LDWEIGHTS pressure
                    yps_ct = [yps.tile([P, 512], F32, tag="yp", name="yp")
                              for _ in range(OO)]
                    for hi in range(HO):
                        for ot in range(OO):
                            nc.tensor.matmul(
                                yps_ct[ot][:],
                                hgs[hi // HG][:, hi % HG, ct * P:(ct + 1) * P],
                                w2c[hi // HC][:, hi % HC, ot * 512:(ot + 1) * 512],
                                start=(hi == 0),
                                stop=(hi == HO - 1),
                            )
                    for ot in range(OO):
                        o_sb = op.tile([P, 512], F32, tag="o")
                        nc.vector.tensor_add(
                            o_sb[:], yps_ct[ot][:], b2_sb[:, ot * 512:(ot + 1) * 512]
                        )
                        n_idx = n0 // P + ct
                        nc.vector.tensor_scalar_mul(
                            o_sb[:], o_sb[:], wt_sb[:, n_idx:n_idx + 1]
                        )
                        nc.sync.dma_start(
                            y_r[:, n_idx, ot * 512:(ot + 1) * 512], o_sb[:]
                        )
    nc.finalize()
    return nc


def kernel(x, W1, b1, W2, b2, gate_w, gate_b):
    x = np.ascontiguousarray(x, dtype=np.float32)
    W1 = np.asarray(W1, dtype=np.float32)
    b1 = np.asarray(b1, dtype=np.float32)
    W2 = np.asarray(W2, dtype=np.float32)
    b2 = np.asarray(b2, dtype=np.float32)
    gate_w = np.ascontiguousarray(gate_w, dtype=np.float32)
    gate_b = np.asarray(gate_b, dtype=np.float32)

    B, D = x.shape
    E, _, H = W1.shape
    O = W2.shape[2]
    assert E == N_CORES and B % (N_CORES * 512) == 0 and D % P == 0
    T = B // N_CORES
    core_ids = list(range(N_CORES))

    # ---- Phase 1: gate logits on device (data-parallel over tokens) ----
    key = ("gate", D, E, T)
    if key not in _build_cache:
        _build_cache[key] = _build_gate(D, E, T)
    nc_gate = _build_cache[key]
    in_maps = [
        {"xT": np.ascontiguousarray(x[i * T:(i + 1) * T].T), "gw": gate_w}
        for i in range(N_CORES)
    ]
    res = run_bass_kernel_spmd(nc_gate, in_maps, core_ids=core_ids)
    logits = np.concatenate(
        [res.results[i]["logitsT"].T for i in range(N_CORES)], axis=0
    ) + gate_b[None, :]

    # ---- Host: top-2 routing (the expert-parallel sharding decision) ----
    lg = logits.astype(np.float64)
    lg -= lg.max(axis=1, keepdims=True)
    probs = np.exp(lg)
    probs /= probs.sum(axis=1, keepdims=True)
    order = np.argsort(-probs, axis=1, kind="stable")[:, :2]
    p_top = np.take_along_axis(probs, order, axis=1)
    w_top = p_top / p_top.sum(axis=1, keepdims=True)  # [B, 2]

    idx_e, wt_e = [], []
    for e in range(E):
        m0 = order[:, 0] == e
        m1 = order[:, 1] == e
        sel = m0 | m1
        idx = np.nonzero(sel)[0]
        w = np.where(m0[sel], w_top[sel, 0], w_top[sel, 1]).astype(np.float32)
        idx_e.append(idx)
        wt_e.append(w)
    max_count = max(len(i) for i in idx_e)
    C = max(CB, ((max_count + P - 1) // P) * P)

    # ---- Phase 2: expert FFN on device (expert-parallel) ----
    key = ("expert", D, H, O, C)
    if key not in _build_cache:
        _build_cache[key] = _build_expert(D, H, O, C)
    nc_exp = _build_cache[key]

    in_maps = []
    for e in range(E):
        n_e = len(idx_e[e])
        xT_pad = np.zeros((D, C), dtype=_bf16_np)
        xT_pad[:, :n_e] = x[idx_e[e]].T.astype(_bf16_np)
        wt_pad = np.zeros(C, dtype=np.float32)
        wt_pad[:n_e] = wt_e[e]
        in_maps.append({
            "xT": xT_pad,
            "w1": W1[e].astype(_bf16_np),
            "w2": W2[e].astype(_bf16_np),
            "b1": np.ascontiguousarray(b1[e].reshape(H // P, P).T),
            "b2": np.ascontiguousarray(np.broadcast_to(b2[e], (P, O))),
            "wt": np.ascontiguousarray(wt_pad.reshape(C // P, P).T),
        })
    res = run_bass_kernel_spmd(nc_exp, in_maps, core_ids=core_ids)

    # ---- Host: un-permute and combine the two expert contributions ----
    out = np.zeros((B, O), dtype=np.float32)
    for e in range(E):
        n_e = len(idx_e[e])
        if n_e:
            out[idx_e[e]] += res.results[e]["y"][:n_e]
    return out
